# revision 6
# baseline (speedup 1.0000x reference)
import numpy as np

# nn_Head: single-head causal attention.
# B=8, T=2048, E=1024, D=128. Data-parallel: one batch element per core.
# Per core: q/k/v projections (bf16), causal softmax(q k^T / sqrt(D)) @ v.
#
# Layout: scores are computed TRANSPOSED (S^T[k, q] = K @ q^T) so the
# post-softmax probabilities come out already in the [k, q] layout the
# PV matmul needs as its stationary operand — no PE transposes at all.
# Softmax skips max-subtraction (scores ~N(0, 1/9), exp can't overflow);
# the denominator falls out of the PV matmul via a ones-column in V.
B, T, E, D = 8, 2048, 1024, 128
SCALE = 1.0 / float(np.sqrt(D))
NT = T // 128   # 16 row tiles
NE = E // 128   # 8 contraction chunks
VE = D + 1      # v columns + ones column (softmax denominator)


def _build():
    from concourse import bacc, bass, tile
    from concourse.bass import mybir

    f32 = mybir.dt.float32
    bf16 = mybir.dt.bfloat16
    AF = mybir.ActivationFunctionType
    nc = bacc.Bacc(None, target_bir_lowering=False)

    XT_d = nc.declare_dram_parameter("XT", [128, NE, T], bf16, isOutput=False)
    Wq_d = nc.declare_dram_parameter("Wq", [128, NE, D], bf16, isOutput=False)
    Wk_d = nc.declare_dram_parameter("Wk", [128, NE, D], bf16, isOutput=False)
    Wv_d = nc.declare_dram_parameter("Wv", [128, NE, D], bf16, isOutput=False)
    tri_d = nc.declare_dram_parameter("tri", [128, 128], bf16, isOutput=False)
    out_d = nc.declare_dram_parameter("out", [T, D], f32, isOutput=True)

    with tile.TileContext(nc) as tc:
        with (
            tc.tile_pool(name="persist", bufs=1) as pp,
            tc.tile_pool(name="work", bufs=4) as wp,
        ):
            XT = pp.tile([128, NE, T], bf16)     # X^T: [e_part, e_chunk, t]
            Wq = pp.tile([128, NE, D], bf16)
            Wk = pp.tile([128, NE, D], bf16)
            Wv = pp.tile([128, NE, D], bf16)
            tri = pp.tile([128, 128], bf16)      # [k, q]: 1 where q >= k
            qT = pp.tile([128, T], bf16)         # q^T [d, t]
            kT = pp.tile([128, T], bf16)         # k^T [d, t]
            v1 = pp.tile([128, NT, VE], bf16)    # v [t, d] + ones col
            PT = pp.tile([128, NT, T], bf16)     # P^T blocks: [k_part, j, q]

            nc.sync.dma_start(Wq[:], Wq_d[:])
            nc.sync.dma_start(Wk[:], Wk_d[:])
            for c in range(8):
                nc.sync.dma_start(
                    XT[:, :, c * 256:(c + 1) * 256],
                    XT_d[:, :, c * 256:(c + 1) * 256])
            nc.sync.dma_start(Wv[:], Wv_d[:])
            nc.sync.dma_start(tri[:], tri_d[:])

            nc.vector.memset(v1[:, :, D:VE], 1.0)

            # q^T/k^T: [d, t] = W^T @ X^T in 512-wide t chunks (DVE casts;
            # ACT is reserved for the exp stream, which is near-critical)
            with tc.tile_pool(name="pqk", bufs=4,
                              space=bass.MemorySpace.PSUM) as qkp:
                for c in range(4):
                    for W_, dst in ((Wq, qT), (Wk, kT)):
                        ps = qkp.tile([128, 512], f32)
                        for e in range(NE):
                            nc.tensor.matmul(
                                ps[:], W_[:, e, :],
                                XT[:, e, c * 512:(c + 1) * 512],
                                start=(e == 0), stop=(e == NE - 1))
                        nc.vector.tensor_copy(
                            dst[:, c * 512:(c + 1) * 512], ps[:])

            with (
                tc.tile_pool(name="pst", bufs=2,
                             space=bass.MemorySpace.PSUM) as stp,
                tc.tile_pool(name="pv", bufs=2,
                             space=bass.MemorySpace.PSUM) as pvp,
                tc.tile_pool(name="pacc", bufs=2,
                             space=bass.MemorySpace.PSUM) as pap,
            ):
                # scores S^T[k, q] + exp, in 1024-wide q chunks
                for c in range(2):
                    c0, c1 = c * 1024, (c + 1) * 1024
                    for j in range(NT):
                        q0 = j * 128
                        if q0 >= c1:
                            break
                        s = max(q0, c0)
                        st = stp.tile([128, 1024], f32)
                        for a0 in range(c0, c1, 512):
                            m0 = max(a0, s)
                            a1 = a0 + 512
                            if m0 >= a1:
                                continue
                            nc.tensor.matmul(
                                st[:, m0 - c0:a1 - c0],
                                kT[:, q0:q0 + 128], qT[:, m0:a1],
                                start=True, stop=True)
                        nc.scalar.activation(
                            PT[:, j, s:c1], st[:, s - c0:c1 - c0],
                            AF.Exp, bias=0.0, scale=SCALE)
                        if c0 <= q0:
                            # diagonal block: zero strictly-lower (k > q)
                            nc.vector.tensor_tensor(
                                PT[:, j, q0:q0 + 128], PT[:, j, q0:q0 + 128],
                                tri[:], op=mybir.AluOpType.mult)

                # v: [t, d] = X @ Wv (DVE casts; ACT is busy with exp)
                for t in range(NT):
                    ps = pvp.tile([128, D], f32)
                    for e in range(NE):
                        nc.tensor.matmul(
                            ps[:], XT[:, e, t * 128:(t + 1) * 128],
                            Wv[:, e, :],
                            start=(e == 0), stop=(e == NE - 1))
                    nc.vector.tensor_copy(v1[:, t, 0:D], ps[:])

                # out[q, d] = (P @ [V | 1]) then divide by the ones-column
                for i in range(NT):
                    acc = pap.tile([128, VE], f32)
                    for j in range(i + 1):
                        nc.tensor.matmul(
                            acc[:], PT[:, j, i * 128:(i + 1) * 128],
                            v1[:, j, :],
                            start=(j == 0), stop=(j == i))
                    rcp = wp.tile([128, 1], f32)
                    nc.vector.reciprocal(rcp[:], acc[:, D:VE])
                    o = wp.tile([128, D], f32)
                    nc.vector.tensor_scalar_mul(o[:], acc[:, 0:D], rcp[:])
                    nc.sync.dma_start(out_d[i * 128:(i + 1) * 128, :], o[:])

    nc.compile()
    return nc


_NC = None
LAST_RESULTS = None


def kernel(X, Wq, Wk, Wv):
    global _NC, LAST_RESULTS
    import ml_dtypes
    from concourse.bass_utils import run_bass_kernel_spmd

    bf16 = ml_dtypes.bfloat16
    if _NC is None:
        _NC = _build()
    X = np.asarray(X, np.float32)

    def wprep(W):
        W = np.asarray(W, np.float32).reshape(NE, 128, D).transpose(1, 0, 2)
        return np.ascontiguousarray(W.astype(bf16))

    tri = (np.arange(128)[None, :] >= np.arange(128)[:, None]).astype(bf16)
    base = {
        "Wq": wprep(Wq), "Wk": wprep(Wk), "Wv": wprep(Wv),
        "tri": np.ascontiguousarray(tri),
    }
    in_maps = []
    for b in range(B):
        XTb = X[b].T.reshape(NE, 128, T).transpose(1, 0, 2).astype(bf16)
        in_maps.append(dict(base, XT=np.ascontiguousarray(XTb)))
    res = run_bass_kernel_spmd(_NC, in_maps, core_ids=list(range(B)))
    LAST_RESULTS = res
    outs = []
    for r in res.results:
        outs.append(np.asarray(r["out"] if isinstance(r, dict) else r))
    return np.stack(outs, 0).reshape(B, T, D)


# revision 10
# speedup vs baseline: 1.2726x; 1.2726x over previous
import numpy as np

# nn_Head: single-head causal attention.
# B=8, T=2048, E=1024, D=128. Data-parallel: one batch element per core.
# Per core: q/k/v projections (bf16), causal softmax(q k^T / sqrt(D)) @ v.
#
# Layout: scores are computed TRANSPOSED (S^T[k, q] = K @ q^T) so the
# post-softmax probabilities come out already in the [k, q] layout the
# PV matmul needs as its stationary operand — no PE transposes at all.
# Softmax skips max-subtraction (scores ~N(0, 1/9), exp can't overflow);
# the denominator falls out of the PV matmul via a ones-column in V.
#
# Scheduling: the v-projection's cheap 128-col matmuls lead (absorbing
# the PE p-state ramp), projections are chunked so the first score
# matmuls (and the ACT exp stream, ~20us) start as early as possible.
B, T, E, D = 8, 2048, 1024, 128
SCALE = 1.0 / float(np.sqrt(D))
NT = T // 128   # 16 row tiles
NE = E // 128   # 8 contraction chunks
VE = D + 1      # v columns + ones column (softmax denominator)


def _build():
    from concourse import bacc, bass, tile
    from concourse.bass import mybir

    f32 = mybir.dt.float32
    bf16 = mybir.dt.bfloat16
    AF = mybir.ActivationFunctionType
    nc = bacc.Bacc(None, target_bir_lowering=False)

    XT_d = nc.declare_dram_parameter("XT", [128, NE, T], bf16, isOutput=False)
    Wq_d = nc.declare_dram_parameter("Wq", [128, NE, D], bf16, isOutput=False)
    Wk_d = nc.declare_dram_parameter("Wk", [128, NE, D], bf16, isOutput=False)
    Wv_d = nc.declare_dram_parameter("Wv", [128, NE, D], bf16, isOutput=False)
    tri_d = nc.declare_dram_parameter("tri", [128, 128], bf16, isOutput=False)
    out_d = nc.declare_dram_parameter("out", [T, D], f32, isOutput=True)

    with tile.TileContext(nc) as tc:
        with (
            tc.tile_pool(name="persist", bufs=1) as pp,
            tc.tile_pool(name="work", bufs=4) as wp,
            tc.tile_pool(name="pproj", bufs=2,
                         space=bass.MemorySpace.PSUM) as prp,
            tc.tile_pool(name="pst", bufs=2,
                         space=bass.MemorySpace.PSUM) as stp,
        ):
            XT = pp.tile([128, NE, T], bf16)     # X^T: [e_part, e_chunk, t]
            Wq = pp.tile([128, NE, D], bf16)
            Wk = pp.tile([128, NE, D], bf16)
            Wv = pp.tile([128, NE, D], bf16)
            tri = pp.tile([128, 128], bf16)      # [k, q]: 1 where q >= k
            qT = pp.tile([128, T], bf16)         # q^T [d, t]
            kT = pp.tile([128, T], bf16)         # k^T [d, t]
            v1 = pp.tile([128, NT, VE], bf16)    # v [t, d] + ones col
            PT = pp.tile([128, NT, T], bf16)     # P^T blocks: [k_part, j, q]

            nc.sync.dma_start(XT[:, :, 0:256], XT_d[:, :, 0:256])
            nc.sync.dma_start(Wv[:], Wv_d[:])
            nc.sync.dma_start(XT[:, :, 256:512], XT_d[:, :, 256:512])
            nc.sync.dma_start(Wq[:], Wq_d[:])
            nc.sync.dma_start(Wk[:], Wk_d[:])
            for c in range(2, 8):
                nc.sync.dma_start(
                    XT[:, :, c * 256:(c + 1) * 256],
                    XT_d[:, :, c * 256:(c + 1) * 256])
            nc.sync.dma_start(tri[:], tri_d[:])

            nc.vector.memset(v1[:, :, D:VE], 1.0)

            def v_proj(t):
                # shares the "sm" bank-pair with pv()'s accumulators
                ps = stp.tile([128, VE], f32, tag="sm")
                for e in range(NE):
                    nc.tensor.matmul(
                        ps[:, 0:D], XT[:, e, t * 128:(t + 1) * 128],
                        Wv[:, e, :],
                        start=(e == 0), stop=(e == NE - 1))
                nc.vector.tensor_copy(v1[:, t, 0:D], ps[:, 0:D])

            def qk_proj(c, W_, dst):
                ps = prp.tile([128, 512], f32, tag="pqk")
                for e in range(NE):
                    nc.tensor.matmul(
                        ps[:], W_[:, e, :], XT[:, e, c * 512:(c + 1) * 512],
                        start=(e == 0), stop=(e == NE - 1))
                nc.vector.tensor_copy(dst[:, c * 512:(c + 1) * 512], ps[:])

            def scores(c0, c1, j):
                # S^T[k, q] for k-block j, q in [max(j*128,c0), c1)
                q0 = j * 128
                s = max(q0, c0)
                st = stp.tile([128, 1024], f32, tag="st")
                for a0 in range(c0, c1, 512):
                    m0 = max(a0, s)
                    a1 = a0 + 512
                    if m0 >= a1:
                        continue
                    nc.tensor.matmul(
                        st[:, m0 - c0:a1 - c0],
                        kT[:, q0:q0 + 128], qT[:, m0:a1],
                        start=True, stop=True)
                nc.scalar.activation(
                    PT[:, j, s:c1], st[:, s - c0:c1 - c0],
                    AF.Exp, bias=0.0, scale=SCALE)
                if c0 <= q0:
                    # diagonal block: zero strictly-lower (k > q)
                    nc.vector.tensor_tensor(
                        PT[:, j, q0:q0 + 128], PT[:, j, q0:q0 + 128],
                        tri[:], op=mybir.AluOpType.mult)

            def pv(i):
                acc = stp.tile([128, VE], f32, tag="sm")
                for j in range(i + 1):
                    nc.tensor.matmul(
                        acc[:], PT[:, j, i * 128:(i + 1) * 128], v1[:, j, :],
                        start=(j == 0), stop=(j == i))
                rcp = wp.tile([128, 1], f32)
                nc.vector.reciprocal(rcp[:], acc[:, D:VE])
                o = wp.tile([128, D], f32)
                nc.vector.tensor_scalar_mul(o[:], acc[:, 0:D], rcp[:])
                nc.sync.dma_start(out_d[i * 128:(i + 1) * 128, :], o[:])

            # v(0..3) leads: cheap 128-col matmuls absorb the p-state ramp
            for t in range(4):
                v_proj(t)
            for c in range(2):
                qk_proj(c, Wq, qT)
            for c in range(2):
                qk_proj(c, Wk, kT)
            for j in range(8):
                scores(0, 1024, j)
            for t in range(4, 8):
                v_proj(t)
            for c in range(2, 4):
                qk_proj(c, Wq, qT)
            for c in range(2, 4):
                qk_proj(c, Wk, kT)
            for j in range(NT):
                scores(1024, 2048, j)
            for t in range(8, NT):
                v_proj(t)
            for i in range(NT):
                pv(i)

    nc.compile()
    return nc


_NC = None
LAST_RESULTS = None


def kernel(X, Wq, Wk, Wv):
    global _NC, LAST_RESULTS
    import ml_dtypes
    from concourse.bass_utils import run_bass_kernel_spmd

    bf16 = ml_dtypes.bfloat16
    if _NC is None:
        _NC = _build()
    X = np.asarray(X, np.float32)

    def wprep(W):
        W = np.asarray(W, np.float32).reshape(NE, 128, D).transpose(1, 0, 2)
        return np.ascontiguousarray(W.astype(bf16))

    tri = (np.arange(128)[None, :] >= np.arange(128)[:, None]).astype(bf16)
    base = {
        "Wq": wprep(Wq), "Wk": wprep(Wk), "Wv": wprep(Wv),
        "tri": np.ascontiguousarray(tri),
    }
    in_maps = []
    for b in range(B):
        XTb = X[b].T.reshape(NE, 128, T).transpose(1, 0, 2).astype(bf16)
        in_maps.append(dict(base, XT=np.ascontiguousarray(XTb)))
    res = run_bass_kernel_spmd(_NC, in_maps, core_ids=list(range(B)))
    LAST_RESULTS = res
    outs = []
    for r in res.results:
        outs.append(np.asarray(r["out"] if isinstance(r, dict) else r))
    return np.stack(outs, 0).reshape(B, T, D)


# revision 22
# speedup vs baseline: 1.2905x; 1.0140x over previous
import numpy as np

# nn_Head: single-head causal attention.
# B=8, T=2048, E=1024, D=128. Data-parallel: one batch element per core.
# Per core: q/k/v projections (bf16), causal softmax(q k^T / sqrt(D)) @ v.
#
# Layout: scores are computed TRANSPOSED (S^T[k, q] = K @ q^T) so the
# post-softmax probabilities come out already in the [k, q] layout the
# PV matmul needs as its stationary operand — no PE transposes at all.
# Softmax skips max-subtraction (scores ~N(0, 1/9), exp can't overflow);
# the denominator falls out of the PV matmul via a ones-column in V.
#
# Scheduling: the v-projection's cheap 128-col matmuls lead (absorbing
# the PE p-state ramp), projections are chunked so the first score
# matmuls (and the ACT exp stream, ~20us) start as early as possible.
B, T, E, D = 8, 2048, 1024, 128
SCALE = 1.0 / float(np.sqrt(D))
NT = T // 128   # 16 row tiles
NE = E // 128   # 8 contraction chunks
VE = D + 1      # v columns + ones column (softmax denominator)


def _build():
    from concourse import bacc, bass, tile
    from concourse.bass import mybir

    f32 = mybir.dt.float32
    bf16 = mybir.dt.bfloat16
    fp8 = mybir.dt.float8e4
    AF = mybir.ActivationFunctionType
    DR = mybir.MatmulPerfMode.DoubleRow
    nc = bacc.Bacc(None, target_bir_lowering=False)

    XT_d = nc.declare_dram_parameter("XT", [128, NE, T], bf16, isOutput=False)
    Wq_d = nc.declare_dram_parameter("Wq", [128, NE, D], bf16, isOutput=False)
    Wk_d = nc.declare_dram_parameter("Wk", [128, NE, D], bf16, isOutput=False)
    Wv_d = nc.declare_dram_parameter("Wv", [128, NE, D], bf16, isOutput=False)
    tri_d = nc.declare_dram_parameter("tri", [128, 128], bf16, isOutput=False)
    out_d = nc.declare_dram_parameter("out", [NT, 128, D], f32, isOutput=True)

    with tile.TileContext(nc) as tc:
        with (
            tc.tile_pool(name="persist", bufs=1) as pp,
            tc.tile_pool(name="work", bufs=4) as wp,
            tc.tile_pool(name="pproj", bufs=2,
                         space=bass.MemorySpace.PSUM) as prp,
            tc.tile_pool(name="pst", bufs=2,
                         space=bass.MemorySpace.PSUM) as stp,
        ):
            XT = pp.tile([128, NE, T], bf16)     # X^T: [e_part, e_chunk, t]
            Wq = pp.tile([128, NE, D], bf16)
            Wk = pp.tile([128, NE, D], bf16)
            Wv = pp.tile([128, NE, D], bf16)
            tri = pp.tile([128, 128], bf16)      # [k, q]: 1 where q >= k
            qT = pp.tile([128, T], bf16)         # q^T [d, t]
            kT = pp.tile([128, T], bf16)         # k^T [d, t]
            v1 = pp.tile([128, NT, VE], bf16)    # v [t, d] + ones col
            PT = pp.tile([128, NT, T], bf16)     # P^T blocks: [k_part, j, q]

            nc.sync.dma_start(XT[:, :, 0:256], XT_d[:, :, 0:256])
            nc.sync.dma_start(Wv[:], Wv_d[:])
            nc.sync.dma_start(XT[:, :, 256:512], XT_d[:, :, 256:512])
            nc.sync.dma_start(Wq[:], Wq_d[:])
            nc.sync.dma_start(Wk[:], Wk_d[:])
            for c in range(2, 8):
                nc.sync.dma_start(
                    XT[:, :, c * 256:(c + 1) * 256],
                    XT_d[:, :, c * 256:(c + 1) * 256])
            nc.sync.dma_start(tri[:], tri_d[:])

            nc.vector.memset(v1[:, :, D:VE], 1.0)

            def v_proj(t):
                # shares the "sm" bank-pair with pv()'s accumulators
                ps = stp.tile([128, VE], f32, tag="sm")
                for e in range(NE):
                    nc.tensor.matmul(
                        ps[:, 0:D], XT[:, e, t * 128:(t + 1) * 128],
                        Wv[:, e, :],
                        start=(e == 0), stop=(e == NE - 1))
                nc.vector.tensor_copy(v1[:, t, 0:D], ps[:, 0:D])

            def qk_proj(c, W_, dst):
                ps = prp.tile([128, 512], f32, tag="pqk")
                for e in range(NE):
                    nc.tensor.matmul(
                        ps[:], W_[:, e, :], XT[:, e, c * 512:(c + 1) * 512],
                        start=(e == 0), stop=(e == NE - 1))
                nc.vector.tensor_copy(dst[:, c * 512:(c + 1) * 512], ps[:])

            def scores(c0, c1, j):
                # S^T[k, q] for k-block j, q in [max(j*128,c0), c1)
                q0 = j * 128
                s = max(q0, c0)
                st = stp.tile([128, 1024], f32, tag="st")
                for a0 in range(c0, c1, 512):
                    m0 = max(a0, s)
                    a1 = a0 + 512
                    if m0 >= a1:
                        continue
                    nc.tensor.matmul(
                        st[:, m0 - c0:a1 - c0],
                        kT[:, q0:q0 + 128], qT[:, m0:a1],
                        start=True, stop=True)
                nc.scalar.activation(
                    PT[:, j, s:c1], st[:, s - c0:c1 - c0],
                    AF.Exp, bias=0.0, scale=SCALE)
                if c0 <= q0:
                    # diagonal block: zero strictly-lower (k > q)
                    nc.vector.tensor_tensor(
                        PT[:, j, q0:q0 + 128], PT[:, j, q0:q0 + 128],
                        tri[:], op=mybir.AluOpType.mult)

            def pv(i, ob):
                acc = stp.tile([128, VE], f32, tag="sm")
                for j in range(i + 1):
                    nc.tensor.matmul(
                        acc[:], PT[:, j, i * 128:(i + 1) * 128], v1[:, j, :],
                        start=(j == 0), stop=(j == i))
                rcp = wp.tile([128, 1], f32)
                nc.vector.reciprocal(rcp[:], acc[:, D:VE])
                nc.vector.tensor_scalar_mul(
                    ob[:, i % 4, :], acc[:, 0:D], rcp[:])

            # v(0..3) leads: cheap 128-col matmuls absorb the p-state ramp
            for t in range(4):
                v_proj(t)
            for c in range(2):
                qk_proj(c, Wq, qT)
            for c in range(2):
                qk_proj(c, Wk, kT)
            for j in range(8):
                scores(0, 1024, j)
            for t in range(4, 8):
                v_proj(t)
            for c in range(2, 4):
                qk_proj(c, Wq, qT)
            for c in range(2, 4):
                qk_proj(c, Wk, kT)
            for j in range(NT):
                scores(1024, 2048, j)
            for t in range(8, NT):
                v_proj(t)
            for g in range(NT // 4):
                ob = wp.tile([128, 4, D], f32, tag="ob")
                for i in range(g * 4, g * 4 + 4):
                    pv(i, ob)
                nc.sync.dma_start(
                    out_d[g * 4:(g + 1) * 4].rearrange("a b c -> b a c"),
                    ob[:])

    nc.compile()
    return nc


_NC = None
LAST_RESULTS = None


def kernel(X, Wq, Wk, Wv):
    global _NC, LAST_RESULTS
    import ml_dtypes
    from concourse.bass_utils import run_bass_kernel_spmd

    bf16 = ml_dtypes.bfloat16
    fp8 = ml_dtypes.float8_e4m3
    if _NC is None:
        _NC = _build()
    X = np.asarray(X, np.float32)

    def wprep(W):
        W = np.asarray(W, np.float32).reshape(NE, 128, D).transpose(1, 0, 2)
        return np.ascontiguousarray(W.astype(bf16))

    tri = (np.arange(128)[None, :] >= np.arange(128)[:, None]).astype(bf16)
    base = {
        "Wq": wprep(Wq), "Wk": wprep(Wk), "Wv": wprep(Wv),
        "tri": np.ascontiguousarray(tri),
    }
    in_maps = []
    for b in range(B):
        XTb = X[b].T.reshape(NE, 128, T).transpose(1, 0, 2).astype(bf16)
        in_maps.append(dict(base, XT=np.ascontiguousarray(XTb)))
    res = run_bass_kernel_spmd(_NC, in_maps, core_ids=list(range(B)))
    LAST_RESULTS = res
    outs = []
    for r in res.results:
        outs.append(np.asarray(r["out"] if isinstance(r, dict) else r))
    return np.stack(outs, 0).reshape(B, T, D)
